# revision 22
# baseline (speedup 1.0000x reference)
"""GQA (grouped-query attention) Trainium2 kernel, 8-core SPMD.

Sharding: core = (batch b, query-quarter q4).  Each core:
  - projects k^T and v only for its OWN 512-token sequence chunk (the same
    columns as its query quarter, so xTq doubles as the kv-proj input),
    then AllGathers the 4 chunks within its batch group -> full-S K/V,
  - projects q^T for its 512-query quarter (overlaps the collective),
  - computes scores^T = k @ q^T per head, exp (fused scale+mask-bias on ACT),
    an AV matmul whose extra ones column yields the softmax denominator,
  - normalizes via DMA partition-broadcast of the reciprocal row,
  - applies out-proj transposed: out^T = o_w^T @ ctx^T.
Host transposes/assembles the quarters back to [B, S, D].

Everything is laid out "transposed" (feature-dim on partitions, sequence on
the free axis) so softmax reductions, biases and the key mask are all
free-axis / per-partition ops and no on-chip transposes are needed.
Matmul data is bf16 (full PE rate); all accumulation is fp32 in PSUM.

Head pairing: PE matmul needs lhsT/rhs at the same base partition.  k^T for
group g lives at partition base (g%2)*64, so q^T tiles pair one even-group
head (partitions 0:64) with one odd-group head (64:128) via a host-side
permutation of q_w columns / o_w rows; the two score matmuls of a pair run
concurrently in separate PE row-tiles.
"""

import numpy as np
import ml_dtypes

import concourse.bass as bass
import concourse.mybir as mybir
import concourse.tile as tile
from concourse import bacc
from concourse import bass_utils

B, S, D = 2, 2048, 2048
H, G = 32, 8
HD = D // H            # 64
HPG = H // G           # 4
KV = G * HD            # 512
P = 128
SQ = S // 4            # 512 queries per core
NK = D // P            # 16 contraction chunks
NSK = S // P           # 16 key tiles
N_CORES = 8

HEADS_E = [h for h in range(H) if (h // HPG) % 2 == 0]
HEADS_O = [h for h in range(H) if (h // HPG) % 2 == 1]

BF16 = mybir.dt.bfloat16
F32 = mybir.dt.float32
EXP = mybir.ActivationFunctionType.Exp
ADD = mybir.AluOpType.add
MULT = mybir.AluOpType.mult

_CACHE = {}


def _build():
    nc = bacc.Bacc("TRN2", target_bir_lowering=False, debug=False,
                   num_devices=N_CORES)

    xTq_d = nc.dram_tensor("xTq", [D, SQ], BF16, kind="ExternalInput")
    qw_d = nc.dram_tensor("qw", [D, D], BF16, kind="ExternalInput")
    kw_d = nc.dram_tensor("kw", [D, KV], BF16, kind="ExternalInput")
    vw_d = nc.dram_tensor("vw", [D, KV], BF16, kind="ExternalInput")
    ow_d = nc.dram_tensor("ow", [D, D], BF16, kind="ExternalInput")
    qb_d = nc.dram_tensor("qb", [D, 1], F32, kind="ExternalInput")
    kb_d = nc.dram_tensor("kb", [KV, 1], F32, kind="ExternalInput")
    ob_d = nc.dram_tensor("ob", [D, 1], F32, kind="ExternalInput")
    mb_d = nc.dram_tensor("mb", [S, 1], F32, kind="ExternalInput")
    vbb_d = nc.dram_tensor("vbb", [P, KV], BF16, kind="ExternalInput")
    von_d = nc.dram_tensor("von", [P, G], BF16, kind="ExternalInput")
    outT_d = nc.dram_tensor("outT", [D, SQ], F32, kind="ExternalOutput")

    with tile.TileContext(nc) as tc:
        with (
            tc.tile_pool(name="resid", bufs=1) as resid,
            tc.tile_pool(name="kvs", bufs=4) as kvs_pool,
            tc.tile_pool(name="qwp", bufs=6) as qw_pool,
            tc.tile_pool(name="owp", bufs=12) as ow_pool,
            tc.tile_pool(name="attn", bufs=6) as attn_pool,
            tc.tile_pool(name="ev", bufs=3) as ev_pool,
            tc.tile_pool(name="bc", bufs=4) as bc_pool,
            tc.tile_pool(name="rc", bufs=4) as rc_pool,
            tc.tile_pool(name="dram", bufs=1, space="DRAM") as dram_pool,
            tc.tile_pool(name="psA", bufs=4, space="PSUM") as psA,
            tc.tile_pool(name="psS", bufs=2, space="PSUM") as psS,
        ):
            # ---- resident loads ----
            # kw/xTq interleaved first: the k-projection's first matmul needs
            # kw[0] and xTq[0], so don't queue all of kw+vw ahead of xTq
            kw_sb = resid.tile([P, NK, KV], BF16)
            vw_sb = resid.tile([P, NK, KV], BF16)
            xTq_sb = resid.tile([P, NK, SQ], BF16)
            for k in range(NK):
                nc.sync.dma_start(kw_sb[:, k, :], kw_d.ap()[k * P:(k + 1) * P, :])
                nc.sync.dma_start(xTq_sb[:, k, :], xTq_d.ap()[k * P:(k + 1) * P, :])
            kb_sb = resid.tile([P, KV // P], F32)
            nc.sync.dma_start(kb_sb[:], kb_d.ap().rearrange("(k p) one -> p (k one)", p=P))
            for k in range(NK):
                nc.sync.dma_start(vw_sb[:, k, :], vw_d.ap()[k * P:(k + 1) * P, :])
            mb_sb = resid.tile([P, NSK], F32)
            nc.sync.dma_start(mb_sb[:], mb_d.ap().rearrange("(i p) one -> p (i one)", p=P))
            vbb_sb = resid.tile([P, G, HD], BF16)
            nc.sync.dma_start(vbb_sb[:], vbb_d.ap().rearrange("p (g d) -> p g d", g=G))
            von_sb = resid.tile([P, G, 1], BF16)
            nc.sync.dma_start(von_sb[:], von_d.ap().rearrange("p (g one) -> p g one", g=G))
            qb_sb = resid.tile([P, NK], F32)
            nc.sync.dma_start(qb_sb[:], qb_d.ap().rearrange("(k p) one -> p (k one)", p=P))
            ob_sb = resid.tile([P, NK], F32)
            nc.sync.dma_start(ob_sb[:], ob_d.ap().rearrange("(k p) one -> p (k one)", p=P))

            # ---- big resident intermediates ----
            kT_all = resid.tile([P, KV // P, 4, SQ], BF16)  # k^T [kv-col, j, t]
            v_ones = resid.tile([P, NSK, G, HD + 1], BF16)  # v (+ones col) per Sk tile
            qT_all = resid.tile([P, NK, SQ], BF16)          # q^T  [q-col, Sq]
            ctx_all = resid.tile([P, NK, SQ], BF16)         # ctx^T stacked head pairs

            # ---- local K/V projection on own 512-seq chunk ----
            # kv_in rows 0:512 = k^T chunk (512 cols); rows 512:1024 = v chunk
            # in [g, d+ones] layout (520 cols) so the softmax-denominator ones
            # column travels through the AllGather and unpacks contiguously.
            VW1 = G * (HD + 1)  # 520
            kv_in = dram_pool.tile([1024, VW1], BF16)
            kv_out = dram_pool.tile([4 * 1024, VW1], BF16)

            for m in range(4):
                ps_k = psA.tile([P, SQ], F32, tag="ps", name=f"ps_k{m}")
                for k in range(NK):
                    nc.tensor.matmul(ps_k[:], kw_sb[:, k, m * P:(m + 1) * P],
                                     xTq_sb[:, k, :], start=(k == 0), stop=(k == NK - 1))
                kt = kvs_pool.tile([P, SQ], BF16, tag="kv")
                nc.vector.tensor_scalar_add(kt[:], ps_k[:], kb_sb[:, m:m + 1])
                # gpsimd queue: keeps the collective's input path off the
                # sync queue, which is busy streaming qw/ow weight tiles
                nc.gpsimd.dma_start(kv_in[m * P:(m + 1) * P, 0:SQ], kt[:])
            for s in range(4):
                ps_v = psA.tile([P, KV], F32, tag="ps", name=f"ps_v{s}")
                for k in range(NK):
                    nc.tensor.matmul(ps_v[:], xTq_sb[:, k, s * P:(s + 1) * P],
                                     vw_sb[:, k, :], start=(k == 0), stop=(k == NK - 1))
                vt = kvs_pool.tile([P, G, HD + 1], BF16, tag="kv")
                nc.vector.tensor_tensor(vt[:, :, 0:HD],
                                        ps_v[:].rearrange("p (g d) -> p g d", g=G),
                                        vbb_sb[:], op=ADD)
                nc.vector.tensor_copy(vt[:, :, HD:HD + 1], von_sb[:])
                nc.gpsimd.dma_start(kv_in[512 + s * P:512 + (s + 1) * P, :],
                                    vt[:].rearrange("p g c -> p (g c)"))

            # ---- AllGather K/V chunks within each batch's 4-core group ----
            nc.gpsimd.collective_compute(
                "AllGather", mybir.AluOpType.bypass,
                replica_groups=[[0, 1, 2, 3], [4, 5, 6, 7]],
                ins=[kv_in.opt()], outs=[kv_out.opt()])

            # ---- q^T projection (overlaps the collective) ----
            # qw is host-blocked as [hp*P+p, k*P+c] = q_w[k*P+p, perm[hp*P+c]]
            # so each tile load is a contiguous 0.5MB block (4KB/partition).
            def emit_qproj(hp):
                qwt = qw_pool.tile([P, NK, P], BF16, tag="qw", name=f"qwt{hp}")
                nc.sync.dma_start(qwt[:], qw_d.ap()[hp * P:(hp + 1) * P, :]
                                  .rearrange("p (k c) -> p k c", c=P))
                ps_q = psA.tile([P, SQ], F32, tag="ps", name=f"ps_q{hp}")
                for k in range(NK):
                    nc.tensor.matmul(ps_q[:], qwt[:, k, :], xTq_sb[:, k, :],
                                     start=(k == 0), stop=(k == NK - 1))
                nc.vector.tensor_scalar_add(qT_all[:, hp, :], ps_q[:],
                                            qb_sb[:, hp:hp + 1])

            for hp in range(NK):
                emit_qproj(hp)

            # ---- unpack gathered K/V (gpsimd queue: ordered after the
            # collective there, and not stuck behind qw loads on sync) ----
            for m in range(4):
                for j in range(4):
                    nc.gpsimd.dma_start(
                        kT_all[:, m, j, :],
                        kv_out[(j * 8 + m) * P:(j * 8 + m + 1) * P, 0:SQ])
            for j in range(4):
                for s in range(4):
                    i = 4 * j + s
                    nc.gpsimd.dma_start(
                        v_ones[:, i, :, :],
                        kv_out[(j * 8 + 4 + s) * P:(j * 8 + 4 + s + 1) * P, :]
                        .rearrange("p (g c) -> p g c", g=G))

            def emit_scores(hp, i):
                gA = HEADS_E[hp] // HPG
                gB = HEADS_O[hp] // HPG
                j, t = i // 4, (i % 4) * P
                sc = psS.tile([P, 2 * SQ], F32, tag="sc", name=f"sc{hp}_{i}")
                nc.tensor.matmul(sc[:, 0:SQ],
                                 kT_all[0:HD, gA // 2, j, t:t + P],
                                 qT_all[0:HD, hp, :], start=True, stop=True)
                nc.tensor.matmul(sc[:, SQ:2 * SQ],
                                 kT_all[HD:2 * HD, gB // 2, j, t:t + P],
                                 qT_all[HD:2 * HD, hp, :], start=True, stop=True)
                return sc

            sc_next = emit_scores(0, 0)
            for hp in range(NK):
                gA = HEADS_E[hp] // HPG
                gB = HEADS_O[hp] // HPG
                ctx0 = psA.tile([P, SQ], F32, tag="ps", name=f"ctx0_{hp}")
                ctx1 = psA.tile([P, SQ], F32, tag="ps", name=f"ctx1_{hp}")
                for i in range(NSK):
                    sc = sc_next
                    at = attn_pool.tile([P, 2 * SQ], BF16, tag="at")
                    nc.scalar.activation(at[:], sc[:], EXP,
                                         bias=mb_sb[:, i:i + 1], scale=0.125)
                    if i + 1 < NSK:
                        sc_next = emit_scores(hp, i + 1)
                    elif hp + 1 < NK:
                        sc_next = emit_scores(hp + 1, 0)
                    nc.tensor.matmul(ctx0[0:HD + 1, :],
                                     v_ones[:, i, gA, :], at[:, 0:SQ],
                                     start=(i == 0), stop=(i == NSK - 1))
                    nc.tensor.matmul(ctx1[0:HD + 1, :],
                                     v_ones[:, i, gB, :], at[:, SQ:2 * SQ],
                                     start=(i == 0), stop=(i == NSK - 1))
                # normalize: ctx[c, q] / Z[q]  (Z = ones-column row)
                for half, cps in ((0, ctx0), (1, ctx1)):
                    rc = rc_pool.tile([1, SQ], F32, tag="rc")
                    nc.vector.reciprocal(rc[:], cps[HD:HD + 1, :])
                    bc = bc_pool.tile([HD, SQ], F32, tag="bc")
                    nc.gpsimd.partition_broadcast(bc[:], rc[:])
                    nc.vector.tensor_tensor(
                        ctx_all[half * HD:(half + 1) * HD, hp, :],
                        cps[0:HD, :], bc[:], op=MULT)

            # ---- out projection (transposed): out^T = o_w^T @ ctx^T ----
            for mg in range(4):
                ps_o = [psA.tile([P, SQ], F32, tag="ps", name=f"ps_o{mg}_{mj}")
                        for mj in range(4)]
                for k in range(NK):
                    owt = ow_pool.tile([P, 512], BF16, tag="ow")
                    nc.sync.dma_start(owt[:], ow_d.ap()[k * P:(k + 1) * P,
                                                        mg * 512:(mg + 1) * 512])
                    for mj in range(4):
                        nc.tensor.matmul(ps_o[mj][:], owt[:, mj * P:(mj + 1) * P],
                                         ctx_all[:, k, :],
                                         start=(k == 0), stop=(k == NK - 1))
                for mj in range(4):
                    m = mg * 4 + mj
                    ot = ev_pool.tile([P, SQ], F32, tag="ot")
                    nc.vector.tensor_scalar_add(ot[:], ps_o[mj][:], ob_sb[:, m:m + 1])
                    nc.sync.dma_start(outT_d.ap()[m * P:(m + 1) * P, :], ot[:])

    nc.compile()
    return nc


def _get_nc():
    if "nc" not in _CACHE:
        _CACHE["nc"] = _build()
    return _CACHE["nc"]


def prep_in_maps(x, mask, q_w, q_b, k_w, k_b, v_w, v_b, o_w, o_b):
    """Host-side sharding: returns the 8 per-core input dicts."""
    bf = ml_dtypes.bfloat16
    x = np.asarray(x, np.float32)
    mask = np.asarray(mask)
    q_w = np.asarray(q_w, np.float32)
    q_b = np.asarray(q_b, np.float32)
    o_w = np.asarray(o_w, np.float32)
    v_b = np.asarray(v_b, np.float32)

    # head permutation: tile hp = (HEADS_E[hp], HEADS_O[hp])
    col_perm = np.zeros(D, np.int64)
    for hp in range(NK):
        col_perm[hp * P:hp * P + HD] = np.arange(HEADS_E[hp] * HD,
                                                 (HEADS_E[hp] + 1) * HD)
        col_perm[hp * P + HD:(hp + 1) * P] = np.arange(HEADS_O[hp] * HD,
                                                       (HEADS_O[hp] + 1) * HD)
    # qw blocked per head-pair tile: qw[hp*P+p, k*P+c] = q_w[k*P+p, perm[hp*P+c]]
    qp = q_w[:, col_perm].reshape(NK, P, NK, P)
    qw = np.ascontiguousarray(qp.transpose(2, 1, 0, 3)).reshape(D, D).astype(bf)
    qb = np.ascontiguousarray(q_b[col_perm]).reshape(D, 1)
    ow = np.ascontiguousarray(o_w[col_perm, :]).astype(bf)

    kw = np.asarray(k_w, np.float32).astype(bf)
    vw = np.asarray(v_w, np.float32).astype(bf)
    kb = np.asarray(k_b, np.float32).reshape(KV, 1)
    ob = np.asarray(o_b, np.float32).reshape(D, 1)

    vbb = np.broadcast_to(v_b[None, :], (P, KV)).astype(bf)
    von = np.ones((P, G), np.float32).astype(bf)

    xT = [np.ascontiguousarray(x[b].T).astype(bf) for b in range(B)]
    mb = [np.where(np.asarray(mask[b]) == 0, np.float32(-30000.0),
                   np.float32(0.0)).reshape(S, 1) for b in range(B)]

    in_maps = []
    for c in range(N_CORES):
        b, q4 = c // 4, c % 4
        in_maps.append({
            "xTq": np.ascontiguousarray(xT[b][:, q4 * SQ:(q4 + 1) * SQ]),
            "qw": qw, "kw": kw, "vw": vw, "ow": ow,
            "qb": qb, "kb": kb, "ob": ob,
            "mb": mb[b], "vbb": vbb, "von": von,
        })
    return in_maps


def kernel(x, mask, q_w, q_b, k_w, k_b, v_w, v_b, o_w, o_b):
    in_maps = prep_in_maps(x, mask, q_w, q_b, k_w, k_b, v_w, v_b, o_w, o_b)
    nc = _get_nc()
    res = bass_utils.run_bass_kernel_spmd(nc, in_maps, core_ids=list(range(N_CORES)))
    out = np.empty((B, S, D), np.float32)
    for c in range(N_CORES):
        b, q4 = c // 4, c % 4
        out[b, q4 * SQ:(q4 + 1) * SQ, :] = res.results[c]["outT"].T
    return out
